# revision 30
# baseline (speedup 1.0000x reference)
"""AttentionBlock Trainium2 kernel: 8-way batch-parallel over 8 NeuronCores.

Reference computation (per batch element b):
    tokens = x[b].reshape(C, N).T                  # [N, C], N=1024, C=512
    qkv    = tokens @ w_proj + b_proj              # [N, 3*512]
    per head h (8 heads, D=64):
        att  = softmax(q_h @ k_h.T / 8, axis=keys) # [N, N]
        res_h = att @ v_h                          # [N, 64]
    out = res @ w_out + b_out + tokens             # [N, C]
    return out.T.reshape(C, 32, 32)

Kernel strategy (per core, one batch element), ScalarE-exp-paced pipeline:
  - qk projection transposed: qkT = w_qk.T @ x -> SBUF [d, tokens] (w_proj
    columns host-permuted; head-pair halves in partitions 0-63 / 64-127)
  - v projection direct (v = x.T @ w_v) with per-head ones column at column
    64+(h%2), so attn@v (M=66) accumulates the softmax denominator of the
    even head on PSUM partition 64 and the odd head on partition 65
  - scores transposed scT[j,i] = k.T @ q (row-packed K=64 pairs), exp on
    ScalarE from PSUM; score PSUM pools single-buffered A/B so ScalarE paces
    while PE interleaves attn@v of the previous pair and projections
  - softmax denominators: DVE sum-row copy -> reciprocal_approx_fast per
    pair -> DRAM-bounce partition-broadcast DMA -> DVE multiply
  - out projection outT = w_out.T @ resT, partially overlapped with the last
    pair's attention; residual+bias fused on DVE
  - PSUM (8 banks): shared 2-buf ring "pp" (4) for projections + attn@v res,
    score pools A/B (2+2); tail out-proj pool reuses A/B's banks
  All matmul operands bf16 (fp32 PSUM accumulation).
"""
import sys
sys.path.insert(0, '/opt/trn_rl_repo')

import numpy as np
import ml_dtypes
from contextlib import ExitStack

B, C, N = 8, 512, 1024
NH, D = 8, 64
INNER = NH * D  # 512
SCALE = D ** -0.5

bf16 = ml_dtypes.bfloat16

_cached_run = None
_cached_nc = None


# ---------------------------------------------------------------- bass kernel
def _build_nc():
    import concourse.bass as bass
    import concourse.tile as tile
    from concourse import bacc, mybir

    f32 = mybir.dt.float32
    b16 = mybir.dt.bfloat16
    ts = bass.ts

    nc = bacc.Bacc("TRN2", target_bir_lowering=False, debug=False)

    xb_d = nc.dram_tensor("xb", [C, N], b16, kind="ExternalInput").ap()
    wqk_d = nc.dram_tensor("wqk", [C, 1024], b16, kind="ExternalInput").ap()
    bqk_d = nc.dram_tensor("bqk", [128, 8], f32, kind="ExternalInput").ap()
    wv_d = nc.dram_tensor("wv", [C, 512], b16, kind="ExternalInput").ap()
    bvb_d = nc.dram_tensor("bvb", [128, 512], f32, kind="ExternalInput").ap()
    wo_d = nc.dram_tensor("wo", [INNER, C], b16, kind="ExternalInput").ap()
    bo_d = nc.dram_tensor("bo", [128, 4], f32, kind="ExternalInput").ap()
    recb_d = nc.dram_tensor("recb", [8, N], f32, kind="Internal").ap()
    out_d = nc.dram_tensor("out", [C, N], f32, kind="ExternalOutput").ap()

    with tile.TileContext(nc) as tc, ExitStack() as ctx:
        sb = ctx.enter_context(tc.tile_pool(name="sb", bufs=1))
        upool = ctx.enter_context(tc.tile_pool(name="up", bufs=1))
        rpool = ctx.enter_context(tc.tile_pool(name="rp", bufs=1))

        warm_sb = sb.tile([128, 512], b16)
        nc.gpsimd.memset(warm_sb[:], 0.0)

        # ---- persistent SBUF tensors (input DMAs spread over 3 queues,
        # first-needed first: xb on sync, wqk head-half on scalar, rest gpsimd)
        xb_sb = sb.tile([128, 4, N], b16)
        xbr = xb_d.rearrange("(kc p) n -> p kc n", p=128)
        nc.sync.dma_start(xb_sb[:, 0, :], xbr[:, 0, :])
        nc.sync.dma_start(xb_sb[:, 1, :], xbr[:, 1, :])
        wqk_sb = sb.tile([128, 4, 1024], b16)
        wqkr = wqk_d.rearrange("(kc p) j -> p kc j", p=128)
        nc.scalar.dma_start(wqk_sb[:, :, 0:256], wqkr[:, :, 0:256])
        nc.scalar.dma_start(xb_sb[:, 2, :], xbr[:, 2, :])
        nc.gpsimd.dma_start(xb_sb[:, 3, :], xbr[:, 3, :])
        nc.scalar.dma_start(wqk_sb[:, :, 256:1024], wqkr[:, :, 256:1024])
        bqk_sb = sb.tile([128, 8], f32)
        nc.sync.dma_start(bqk_sb[:], bqk_d[:])
        wv_sb = sb.tile([128, 4, 512], b16)
        nc.gpsimd.dma_start(wv_sb[:], wv_d.rearrange("(kc p) j -> p kc j", p=128))
        bvb_sb = sb.tile([128, 512], f32)
        nc.gpsimd.dma_start(bvb_sb[:], bvb_d[:])
        wo_sb = sb.tile([128, 4, 512], b16)
        nc.sync.dma_start(wo_sb[:], wo_d.rearrange("(kc p) c -> p kc c", p=128))
        bo_sb = sb.tile([128, 4], f32)
        nc.sync.dma_start(bo_sb[:], bo_d[:])

        qkT_sb = sb.tile([128, 8, N], b16)       # [inner%128, qk chunk, token]
        v_sb = sb.tile([128, 8, 8, 96], b16)     # [tok%128, tchunk, h, d|ones|0s]
        resT_sb = sb.tile([128, 4, N], b16)      # [inner%128, pair, token]
        final_sb = sb.tile([128, 4, N], f32)     # [c%128, cchunk, token]
        resU_sb = sb.tile([128, 8, N], b16)      # unnormalized res + sum rows
        tp1_sb = sb.tile([128, 2, N], b16)       # stream-transposed sums
        tp2_sb = sb.tile([128, 2, N], f32)       # strided reciprocals
        tp3_sb = sb.tile([128, 2, N], f32)       # recip rows (back-transposed)
        nc.vector.memset(tp1_sb[:], 1.0)
        nc.vector.memset(tp2_sb[:], 1.0)

        nc.vector.memset(v_sb[:], 0.0)
        nc.vector.memset(v_sb[:, :, :, 64:66], 1.0)  # ones cols -> sum rows
        for cc in range(4):  # final = x + b_out (residual+bias prefill)
            nc.vector.tensor_scalar_add(
                final_sb[:, cc, :], xb_sb[:, cc, :], bo_sb[:, cc, None])

        # ------------------------------------------------ emission helpers
        def qk_chunk(pp, m):
            ps = pp.tile([128, 2, 512], f32, tag="pp", name=f"qk{m}")
            for ih in range(2):
                for kc in range(4):
                    nc.tensor.matmul(
                        ps[:, ih, :],
                        lhsT=wqk_sb[:, kc, ts(m, 128)],
                        rhs=xb_sb[:, kc, ts(ih, 512)],
                        start=(kc == 0), stop=(kc == 3))
            nc.vector.tensor_scalar_add(
                qkT_sb[:, m, :], ps.rearrange("p a b -> p (a b)"),
                bqk_sb[:, m, None])

        def v_chunk(pp, c2):
            ps = pp.tile([128, 2, 512], f32, tag="pp", name=f"v{c2}")
            for half in range(2):
                tch = 2 * c2 + half
                for kc in range(4):
                    nc.tensor.matmul(
                        ps[:, half, :],
                        lhsT=xb_sb[:, kc, ts(tch, 128)],
                        rhs=wv_sb[:, kc, :],
                        start=(kc == 0), stop=(kc == 3))
            for half in range(2):
                nc.vector.tensor_add(
                    v_sb[:, 2 * c2 + half, :, 0:64],
                    ps[:, half, :].rearrange("p (h d) -> p h d", d=64),
                    bvb_sb.rearrange("p (h d) -> p h d", d=64))

        def score_step(scA, scB, t, jc, uA, uB):
            """Score matmuls + exp for (pair t, key chunk jc)."""
            qc, kc = 2 * t, 2 * t + 1
            sA = scA.tile([128, 2, 512], f32, tag="scA", bufs=1,
                          name=f"sA{t}_{jc}")
            sB = scB.tile([128, 2, 512], f32, tag="scB", bufs=1,
                          name=f"sB{t}_{jc}")
            for ih in range(2):
                nc.tensor.matmul(
                    sA[:, ih, :],
                    lhsT=qkT_sb[0:64, kc, ts(jc, 128)],
                    rhs=qkT_sb[0:64, qc, ts(ih, 512)],
                    start=True, stop=True)
            for ih in range(2):
                nc.tensor.matmul(
                    sB[:, ih, :],
                    lhsT=qkT_sb[64:128, kc, ts(jc, 128)],
                    rhs=qkT_sb[64:128, qc, ts(ih, 512)],
                    start=True, stop=True)
            nc.scalar.activation(
                uA[:, jc, :], sA.rearrange("p a b -> p (a b)"),
                mybir.ActivationFunctionType.Exp)
            nc.scalar.activation(
                uB[:, jc, :], sB.rearrange("p a b -> p (a b)"),
                mybir.ActivationFunctionType.Exp)

        def av_step(pp, t, jc, uA, uB, res2):
            """attn@v accumulation matmuls for pair t, key chunk jc."""
            for half in range(2):
                h = 2 * t + half
                u = uA if half == 0 else uB
                if jc == 0:
                    res2[half] = pp.tile([128, 2, 512], f32, tag="pp",
                                         name=f"res{h}")
                for ih in range(2):
                    nc.tensor.matmul(
                        res2[half][0:96, ih, :],
                        lhsT=v_sb[:, jc, h, :],
                        rhs=u[:, jc, ts(ih, 512)],
                        start=(jc == 0), stop=(jc == 7))

        def evict_pair(t, res2):
            """Evict raw res (with sum rows) to SBUF, freeing the PSUM ring."""
            for half in range(2):
                h = 2 * t + half
                nc.vector.tensor_copy(
                    resU_sb[0:96, h, :],
                    res2[half][0:96].rearrange("p a b -> p (a b)"))

        def normalize_pair(t, use_scalar=False):
            """Reciprocal of the pair's sum rows, broadcast, multiply."""
            if use_scalar:
                # ScalarE exp(-ln(s)) straight onto the sum rows (fast tail)
                nc.scalar.activation(
                    tp2_sb[64:65, :, :].rearrange("p a b -> p (a b)"),
                    resU_sb[64:65, 2 * t:2 * t + 2, :].rearrange(
                        "p a b -> p (a b)"),
                    mybir.ActivationFunctionType.Ln)
                nc.scalar.activation(
                    tp3_sb[64:65, :, :].rearrange("p a b -> p (a b)"),
                    tp2_sb[64:65, :, :].rearrange("p a b -> p (a b)"),
                    mybir.ActivationFunctionType.Exp, scale=-1.0)
            else:
                # DVE 32x32 stream transpose -> strided reciprocal -> back
                nc.vector.transpose(
                    tp1_sb[64:96, :, :], resU_sb[64:96, 2 * t:2 * t + 2, :])
                nc.vector.reciprocal(
                    tp2_sb[64:96, :, :].rearrange(
                        "p a (b o) -> p a b o", o=32)[:, :, :, 0],
                    tp1_sb[64:96, :, :].rearrange(
                        "p a (b o) -> p a b o", o=32)[:, :, :, 0])
                nc.vector.transpose(tp3_sb[64:96, :, :], tp2_sb[64:96, :, :])
            for half in range(2):
                h = 2 * t + half
                nc.gpsimd.dma_start(recb_d[h:h + 1, :], tp3_sb[64:65, half, :])
                bc = rpool.tile([64, N], f32, tag="bc", bufs=4, name=f"bc{h}")
                nc.gpsimd.dma_start(
                    bc[:], recb_d[h:h + 1, :].broadcast_to([64, N]))
                if half == 0:
                    nc.vector.tensor_mul(
                        resT_sb[0:64, t, :], resU_sb[0:64, h, :], bc[:])
                else:
                    tmp = rpool.tile([64, N], b16, tag="tmpod", bufs=2,
                                     name=f"tm{h}")
                    nc.vector.tensor_mul(
                        tmp[:], resU_sb[0:64, h, :], bc[:])
                    nc.sync.dma_start(resT_sb[64:128, t, :], tmp[:])

        def out_chunk(op, cc, kcs, psd):
            """Out-projection accumulation over pair chunks kcs for chunk cc."""
            if 0 in kcs:
                psd[cc] = op.tile([128, 2, 512], f32, tag="op", name=f"o{cc}")
            for ih in range(2):
                for kc in kcs:
                    nc.tensor.matmul(
                        psd[cc][:, ih, :],
                        lhsT=wo_sb[:, kc, ts(cc, 128)],
                        rhs=resT_sb[:, kc, ts(ih, 512)],
                        start=(kc == 0), stop=(kc == 3))

        def out_finish(cc, psd):
            nc.vector.tensor_add(
                final_sb[:, cc, :],
                psd[cc].rearrange("p a b -> p (a b)"),
                final_sb[:, cc, :])
            eng = nc.sync if cc % 2 == 0 else nc.gpsimd
            eng.dma_start(
                out_d.rearrange("(cc p) n -> p cc n", p=128)[:, cc, :],
                final_sb[:, cc, :])

        # ------------------------------------------------ pipeline emission
        u_tiles = {}
        res_pairs = {}
        with tc.tile_pool(name="pp", bufs=2, space="PSUM") as pp:
            # HAM warm-up: dummy matmuls keep the PE busy during input DMAs
            # so the qk chunks run at 2.4 GHz (output never read)
            wps = pp.tile([128, 2, 512], f32, tag="pp", name="warm")
            for i in range(18):
                nc.tensor.matmul(wps[:, i % 2, :], lhsT=warm_sb[:, 0:128],
                                 rhs=warm_sb[:], start=True, stop=True)
            # lead-in: the two qk chunks pair 0 needs
            qk_chunk(pp, 0)
            qk_chunk(pp, 1)
            # proj work to interleave into the score loops: pair 0 gets
            # qk2,3 + v0-3; pairs 1/2 get qk4,5 / qk6,7 (headroom there)
            proj_sched = {
                0: [lambda m=m: qk_chunk(pp, m) for m in (2, 3)]
                   + [lambda c2=c2: v_chunk(pp, c2) for c2 in range(4)],
                1: [lambda m=m: qk_chunk(pp, m) for m in (4, 5)],
                2: [lambda m=m: qk_chunk(pp, m) for m in (6, 7)],
                3: [],
            }

            with tc.tile_pool(name="scA", bufs=1, space="PSUM") as scA, \
                 tc.tile_pool(name="scB", bufs=1, space="PSUM") as scB:
                for t in range(4):
                    uA = upool.tile([128, 8, N], b16, tag="U", bufs=4,
                                    name=f"u{2 * t}")
                    uB = upool.tile([128, 8, N], b16, tag="U", bufs=4,
                                    name=f"u{2 * t + 1}")
                    u_tiles[t] = (uA, uB)
                    res_pairs[t] = [None, None]
                    sched = proj_sched[t]
                    slots = (range(len(sched)) if t == 0
                             else [3, 6][:len(sched)])
                    for jc in range(8):
                        score_step(scA, scB, t, jc, uA, uB)
                        if jc in slots:
                            sched[slots.index(jc) if t else jc]()
                        if t > 0:
                            puA, puB = u_tiles[t - 1]
                            av_step(pp, t - 1, jc, puA, puB, res_pairs[t - 1])
                    if t > 0:
                        evict_pair(t - 1, res_pairs[t - 1])
                        normalize_pair(t - 1)

            # pair 3 attention + out projection (op reuses scA/scB's banks).
            # av3 runs first so the PE never stalls on the op-pool bank WAR
            # (out matmuls wait for the last score ACTs).
            with tc.tile_pool(name="op", bufs=2, space="PSUM") as op:
                psd = {}
                uA, uB = u_tiles[3]
                for jc in range(8):
                    av_step(pp, 3, jc, uA, uB, res_pairs[3])
                evict_pair(3, res_pairs[3])
                out_chunk(op, 0, [0, 1, 2], psd)
                out_chunk(op, 1, [0, 1, 2], psd)
                normalize_pair(3, use_scalar=True)
                out_chunk(op, 0, [3], psd)
                out_finish(0, psd)
                out_chunk(op, 1, [3], psd)
                out_finish(1, psd)
                out_chunk(op, 2, [0, 1, 2, 3], psd)
                out_finish(2, psd)
                out_chunk(op, 3, [0, 1, 2, 3], psd)
                out_finish(3, psd)

    nc.compile()
    return nc


# ------------------------------------------------------------- SPMD dispatch
def _make_spmd_fn(nc, n_cores):
    """bass NEFF runner over axon PJRT WITHOUT buffer donation (donation
    hangs the axon backend)."""
    import jax
    import jax.core
    from jax.sharding import Mesh, PartitionSpec
    from jax.experimental.shard_map import shard_map
    from concourse import mybir
    from concourse.bass2jax import _bass_exec_p, install_neuronx_cc_hook

    install_neuronx_cc_hook()

    partition_name = nc.partition_id_tensor.name if nc.partition_id_tensor else None
    in_names, out_names, out_avals = [], [], []
    for alloc in nc.m.functions[0].allocations:
        if not isinstance(alloc, mybir.MemoryLocationSet):
            continue
        name = alloc.memorylocations[0].name
        if alloc.kind == "ExternalInput":
            if name != partition_name:
                in_names.append(name)
        elif alloc.kind == "ExternalOutput":
            out_names.append(name)
            out_avals.append(jax.core.ShapedArray(
                tuple(alloc.tensor_shape), mybir.dt.np(alloc.dtype)))

    n_params = len(in_names)
    all_in_names = list(in_names) + list(out_names)
    if partition_name is not None:
        all_in_names.append(partition_name)
    zero_outs = [np.zeros(a.shape, a.dtype) for a in out_avals]

    def _body(*args):
        operands = list(args)
        if partition_name is not None:
            from concourse.bass2jax import partition_id_tensor
            operands.append(partition_id_tensor())
        return tuple(_bass_exec_p.bind(
            *operands,
            out_avals=tuple(out_avals),
            in_names=tuple(all_in_names),
            out_names=tuple(out_names),
            lowering_input_output_aliases=(),
            sim_require_finite=True,
            sim_require_nnan=True,
            nc=nc,
        ))

    devices = jax.devices()[:n_cores]
    mesh = Mesh(np.asarray(devices), ("core",))
    sharded = jax.jit(
        shard_map(_body, mesh=mesh,
                  in_specs=(PartitionSpec("core"),) * (n_params + len(out_names)),
                  out_specs=(PartitionSpec("core"),) * len(out_names),
                  check_rep=False),
        keep_unused=True)

    def run(in_maps):
        per_core = [[np.asarray(m[k]) for k in in_names] for m in in_maps]
        concat = [np.concatenate([per_core[c][i] for c in range(n_cores)], axis=0)
                  for i in range(n_params)]
        concat += [np.concatenate([z] * n_cores, axis=0) for z in zero_outs]
        outs = [np.asarray(o) for o in sharded(*concat)]
        results = []
        for c in range(n_cores):
            m = {}
            for i, name in enumerate(out_names):
                rows = out_avals[i].shape[0]
                m[name] = outs[i][c * rows:(c + 1) * rows]
            results.append(m)
        return results

    return run


# ------------------------------------------------------------------ host prep
def _prep_weights(w_proj, b_proj, w_out, b_out):
    # permuted qk columns: chunk m (128 cols): pair t=m//2; m even -> q, odd -> k
    perm = np.empty(1024, np.int64)
    scale = np.empty(1024, np.float32)
    for m in range(8):
        t, is_k = m // 2, m % 2
        for p in range(128):
            h = 2 * t + (1 if p >= 64 else 0)
            d = p % 64
            perm[m * 128 + p] = h * 192 + 64 * is_k + d
            scale[m * 128 + p] = 1.0 if is_k else SCALE
    wqk = (w_proj[:, perm] * scale[None, :]).astype(bf16)
    bqk = (b_proj[perm] * scale).astype(np.float32).reshape(8, 128).T.copy()

    vperm = np.array([(j // 64) * 192 + 128 + (j % 64) for j in range(512)],
                     np.int64)
    wv = w_proj[:, vperm].astype(bf16)
    bvb = np.broadcast_to(b_proj[vperm].astype(np.float32), (128, 512)).copy()

    wo = w_out.astype(bf16)
    bo = b_out.astype(np.float32).reshape(4, 128).T.copy()
    return wqk, bqk, wv, bvb, wo, bo


def kernel(x, w_proj, b_proj, w_out, b_out):
    global _cached_run
    x = np.asarray(x, np.float32)
    w_proj = np.asarray(w_proj, np.float32)
    b_proj = np.asarray(b_proj, np.float32)
    w_out = np.asarray(w_out, np.float32)
    b_out = np.asarray(b_out, np.float32)

    global _cached_nc
    if _cached_run is None:
        nc = _build_nc()
        _cached_nc = nc
        _cached_run = _make_spmd_fn(nc, B)

    wqk, bqk, wv, bvb, wo, bo = _prep_weights(w_proj, b_proj, w_out, b_out)
    in_maps = []
    for b in range(B):
        x2d = np.ascontiguousarray(x[b].reshape(C, N))
        in_maps.append(dict(
            xb=x2d.astype(bf16), wqk=wqk, bqk=bqk,
            wv=wv, bvb=bvb, wo=wo, bo=bo))

    res = _cached_run(in_maps)
    out = np.stack([res[b]["out"].reshape(C, 32, 32) for b in range(B)])
    return out.astype(np.float32)


# revision 31
# speedup vs baseline: 1.1721x; 1.1721x over previous
"""AttentionBlock Trainium2 kernel: 8-way batch-parallel over 8 NeuronCores.

Reference computation (per batch element b):
    tokens = x[b].reshape(C, N).T                  # [N, C], N=1024, C=512
    qkv    = tokens @ w_proj + b_proj              # [N, 3*512]
    per head h (8 heads, D=64):
        att  = softmax(q_h @ k_h.T / 8, axis=keys) # [N, N]
        res_h = att @ v_h                          # [N, 64]
    out = res @ w_out + b_out + tokens             # [N, C]
    return out.T.reshape(C, 32, 32)

Kernel strategy (per core, one batch element), ScalarE-exp-paced pipeline:
  - qk projection transposed: qkT = w_qk.T @ x -> SBUF [d, tokens] (w_proj
    columns host-permuted; head-pair halves in partitions 0-63 / 64-127)
  - v projection direct (v = x.T @ w_v) with per-head ones column at column
    64+(h%2), so attn@v (M=66) accumulates the softmax denominator of the
    even head on PSUM partition 64 and the odd head on partition 65
  - scores transposed scT[j,i] = k.T @ q (row-packed K=64 pairs), exp on
    ScalarE from PSUM; score PSUM pools single-buffered A/B so ScalarE paces
    while PE interleaves attn@v of the previous pair and projections
  - softmax denominators: DVE sum-row copy -> reciprocal_approx_fast per
    pair -> DRAM-bounce partition-broadcast DMA -> DVE multiply
  - out projection outT = w_out.T @ resT, partially overlapped with the last
    pair's attention; residual+bias fused on DVE
  - PSUM (8 banks): shared 2-buf ring "pp" (4) for projections + attn@v res,
    score pools A/B (2+2); tail out-proj pool reuses A/B's banks
  All matmul operands bf16 (fp32 PSUM accumulation).
"""
import sys
sys.path.insert(0, '/opt/trn_rl_repo')

import numpy as np
import ml_dtypes
from contextlib import ExitStack

B, C, N = 8, 512, 1024
NH, D = 8, 64
INNER = NH * D  # 512
SCALE = D ** -0.5

bf16 = ml_dtypes.bfloat16

_cached_run = None
_cached_nc = None


# ---------------------------------------------------------------- bass kernel
def _build_nc():
    import concourse.bass as bass
    import concourse.tile as tile
    from concourse import bacc, mybir

    f32 = mybir.dt.float32
    b16 = mybir.dt.bfloat16
    ts = bass.ts

    nc = bacc.Bacc("TRN2", target_bir_lowering=False, debug=False)

    xb_d = nc.dram_tensor("xb", [C, N], b16, kind="ExternalInput").ap()
    wqk_d = nc.dram_tensor("wqk", [C, 1024], b16, kind="ExternalInput").ap()
    bqk_d = nc.dram_tensor("bqk", [128, 8], f32, kind="ExternalInput").ap()
    wv_d = nc.dram_tensor("wv", [C, 512], b16, kind="ExternalInput").ap()
    bvb_d = nc.dram_tensor("bvb", [128, 512], f32, kind="ExternalInput").ap()
    wo_d = nc.dram_tensor("wo", [INNER, C], b16, kind="ExternalInput").ap()
    bo_d = nc.dram_tensor("bo", [128, 4], f32, kind="ExternalInput").ap()
    recb_d = nc.dram_tensor("recb", [8, N], f32, kind="Internal").ap()
    out_d = nc.dram_tensor("out", [C, N], f32, kind="ExternalOutput").ap()

    with tile.TileContext(nc) as tc, ExitStack() as ctx:
        sb = ctx.enter_context(tc.tile_pool(name="sb", bufs=1))
        upool = ctx.enter_context(tc.tile_pool(name="up", bufs=1))
        rpool = ctx.enter_context(tc.tile_pool(name="rp", bufs=1))

        warm_sb = sb.tile([128, 512], b16)
        nc.gpsimd.memset(warm_sb[:], 0.0)

        # ---- persistent SBUF tensors (input DMAs spread over 3 queues,
        # first-needed first: xb on sync, wqk head-half on scalar, rest gpsimd)
        xb_sb = sb.tile([128, 4, N], b16)
        xbr = xb_d.rearrange("(kc p) n -> p kc n", p=128)
        nc.sync.dma_start(xb_sb[:, 0, :], xbr[:, 0, :])
        nc.sync.dma_start(xb_sb[:, 1, :], xbr[:, 1, :])
        wqk_sb = sb.tile([128, 4, 1024], b16)
        wqkr = wqk_d.rearrange("(kc p) j -> p kc j", p=128)
        nc.scalar.dma_start(wqk_sb[:, :, 0:256], wqkr[:, :, 0:256])
        nc.scalar.dma_start(xb_sb[:, 2, :], xbr[:, 2, :])
        nc.gpsimd.dma_start(xb_sb[:, 3, :], xbr[:, 3, :])
        nc.scalar.dma_start(wqk_sb[:, :, 256:1024], wqkr[:, :, 256:1024])
        bqk_sb = sb.tile([128, 8], f32)
        nc.sync.dma_start(bqk_sb[:], bqk_d[:])
        wv_sb = sb.tile([128, 4, 512], b16)
        nc.gpsimd.dma_start(wv_sb[:], wv_d.rearrange("(kc p) j -> p kc j", p=128))
        bvb_sb = sb.tile([128, 512], f32)
        nc.gpsimd.dma_start(bvb_sb[:], bvb_d[:])
        wo_sb = sb.tile([128, 4, 512], b16)
        nc.sync.dma_start(wo_sb[:], wo_d.rearrange("(kc p) c -> p kc c", p=128))
        bo_sb = sb.tile([128, 4], f32)
        nc.sync.dma_start(bo_sb[:], bo_d[:])

        qkT_sb = sb.tile([128, 8, N], b16)       # [inner%128, qk chunk, token]
        v_sb = sb.tile([128, 8, 8, 96], b16)     # [tok%128, tchunk, h, d|ones|0s]
        resT_sb = sb.tile([128, 4, N], b16)      # [inner%128, pair, token]
        final_sb = sb.tile([128, 4, N], f32)     # [c%128, cchunk, token]
        resU_sb = sb.tile([128, 8, N], b16)      # unnormalized res + sum rows
        tp1_sb = sb.tile([128, 2, N], b16)       # stream-transposed sums
        tp2_sb = sb.tile([128, 2, N], f32)       # strided reciprocals
        tp3_sb = sb.tile([128, 2, N], f32)       # recip rows (back-transposed)
        nc.vector.memset(tp1_sb[:], 1.0)
        nc.vector.memset(tp2_sb[:], 1.0)

        nc.vector.memset(v_sb[:], 0.0)
        nc.vector.memset(v_sb[:, :, :, 64:66], 1.0)  # ones cols -> sum rows
        for cc in range(4):  # final = x + b_out (residual+bias prefill)
            nc.vector.tensor_scalar_add(
                final_sb[:, cc, :], xb_sb[:, cc, :], bo_sb[:, cc, None])

        # ------------------------------------------------ emission helpers
        def qk_chunk(pp, m):
            ps = pp.tile([128, 2, 512], f32, tag="pp", name=f"qk{m}")
            for ih in range(2):
                for kc in range(4):
                    nc.tensor.matmul(
                        ps[:, ih, :],
                        lhsT=wqk_sb[:, kc, ts(m, 128)],
                        rhs=xb_sb[:, kc, ts(ih, 512)],
                        start=(kc == 0), stop=(kc == 3))
            nc.vector.tensor_scalar_add(
                qkT_sb[:, m, :], ps.rearrange("p a b -> p (a b)"),
                bqk_sb[:, m, None])

        def v_chunk(pp, c2):
            ps = pp.tile([128, 2, 512], f32, tag="pp", name=f"v{c2}")
            for half in range(2):
                tch = 2 * c2 + half
                for kc in range(4):
                    nc.tensor.matmul(
                        ps[:, half, :],
                        lhsT=xb_sb[:, kc, ts(tch, 128)],
                        rhs=wv_sb[:, kc, :],
                        start=(kc == 0), stop=(kc == 3))
            for half in range(2):
                nc.vector.tensor_add(
                    v_sb[:, 2 * c2 + half, :, 0:64],
                    ps[:, half, :].rearrange("p (h d) -> p h d", d=64),
                    bvb_sb.rearrange("p (h d) -> p h d", d=64))

        def score_step(scA, scB, t, jc, uA, uB):
            """Score matmuls + exp for (pair t, key chunk jc)."""
            qc, kc = 2 * t, 2 * t + 1
            sA = scA.tile([128, 2, 512], f32, tag="scA", bufs=1,
                          name=f"sA{t}_{jc}")
            sB = scB.tile([128, 2, 512], f32, tag="scB", bufs=1,
                          name=f"sB{t}_{jc}")
            for ih in range(2):  # A/B interleaved: disjoint row groups overlap
                nc.tensor.matmul(
                    sA[:, ih, :],
                    lhsT=qkT_sb[0:64, kc, ts(jc, 128)],
                    rhs=qkT_sb[0:64, qc, ts(ih, 512)],
                    start=True, stop=True)
                nc.tensor.matmul(
                    sB[:, ih, :],
                    lhsT=qkT_sb[64:128, kc, ts(jc, 128)],
                    rhs=qkT_sb[64:128, qc, ts(ih, 512)],
                    start=True, stop=True)
            nc.scalar.activation(
                uA[:, jc, :], sA.rearrange("p a b -> p (a b)"),
                mybir.ActivationFunctionType.Exp)
            nc.scalar.activation(
                uB[:, jc, :], sB.rearrange("p a b -> p (a b)"),
                mybir.ActivationFunctionType.Exp)

        def av_step(pp, t, jc, uA, uB, res2):
            """attn@v accumulation matmuls for pair t, key chunk jc."""
            for half in range(2):
                h = 2 * t + half
                u = uA if half == 0 else uB
                if jc == 0:
                    res2[half] = pp.tile([128, 2, 512], f32, tag="pp",
                                         name=f"res{h}")
                for ih in range(2):
                    nc.tensor.matmul(
                        res2[half][0:96, ih, :],
                        lhsT=v_sb[:, jc, h, :],
                        rhs=u[:, jc, ts(ih, 512)],
                        start=(jc == 0), stop=(jc == 7))

        def evict_pair(t, res2):
            """Evict raw res (with sum rows) to SBUF, freeing the PSUM ring."""
            for half in range(2):
                h = 2 * t + half
                nc.vector.tensor_copy(
                    resU_sb[0:96, h, :],
                    res2[half][0:96].rearrange("p a b -> p (a b)"))

        def normalize_pair(t, use_scalar=False):
            """Reciprocal of the pair's sum rows, broadcast, multiply."""
            if use_scalar:
                # ScalarE exp(-ln(s)) straight onto the sum rows (fast tail)
                nc.scalar.activation(
                    tp2_sb[64:65, :, :].rearrange("p a b -> p (a b)"),
                    resU_sb[64:65, 2 * t:2 * t + 2, :].rearrange(
                        "p a b -> p (a b)"),
                    mybir.ActivationFunctionType.Ln)
                nc.scalar.activation(
                    tp3_sb[64:65, :, :].rearrange("p a b -> p (a b)"),
                    tp2_sb[64:65, :, :].rearrange("p a b -> p (a b)"),
                    mybir.ActivationFunctionType.Exp, scale=-1.0)
            else:
                # DVE 32x32 stream transpose -> strided reciprocal -> back
                nc.vector.transpose(
                    tp1_sb[64:96, :, :], resU_sb[64:96, 2 * t:2 * t + 2, :])
                nc.vector.reciprocal(
                    tp2_sb[64:96, :, :].rearrange(
                        "p a (b o) -> p a b o", o=32)[:, :, :, 0],
                    tp1_sb[64:96, :, :].rearrange(
                        "p a (b o) -> p a b o", o=32)[:, :, :, 0])
                nc.vector.transpose(tp3_sb[64:96, :, :], tp2_sb[64:96, :, :])
            for half in range(2):
                h = 2 * t + half
                nc.gpsimd.dma_start(recb_d[h:h + 1, :], tp3_sb[64:65, half, :])
                bc = rpool.tile([64, N], f32, tag="bc", bufs=4, name=f"bc{h}")
                nc.gpsimd.dma_start(
                    bc[:], recb_d[h:h + 1, :].broadcast_to([64, N]))
                if half == 0:
                    nc.vector.tensor_mul(
                        resT_sb[0:64, t, :], resU_sb[0:64, h, :], bc[:])
                else:
                    tmp = rpool.tile([64, N], b16, tag="tmpod", bufs=2,
                                     name=f"tm{h}")
                    nc.vector.tensor_mul(
                        tmp[:], resU_sb[0:64, h, :], bc[:])
                    nc.sync.dma_start(resT_sb[64:128, t, :], tmp[:])

        def out_chunk(op, cc, kcs, psd):
            """Out-projection accumulation over pair chunks kcs for chunk cc."""
            if 0 in kcs:
                psd[cc] = op.tile([128, 2, 512], f32, tag="op", name=f"o{cc}")
            for ih in range(2):
                for kc in kcs:
                    nc.tensor.matmul(
                        psd[cc][:, ih, :],
                        lhsT=wo_sb[:, kc, ts(cc, 128)],
                        rhs=resT_sb[:, kc, ts(ih, 512)],
                        start=(kc == 0), stop=(kc == 3))

        def out_finish(cc, psd):
            nc.vector.tensor_add(
                final_sb[:, cc, :],
                psd[cc].rearrange("p a b -> p (a b)"),
                final_sb[:, cc, :])
            eng = nc.sync if cc % 2 == 0 else nc.gpsimd
            eng.dma_start(
                out_d.rearrange("(cc p) n -> p cc n", p=128)[:, cc, :],
                final_sb[:, cc, :])

        # ------------------------------------------------ pipeline emission
        u_tiles = {}
        res_pairs = {}
        with tc.tile_pool(name="pp", bufs=2, space="PSUM") as pp:
            # HAM warm-up: dummy matmuls keep the PE busy during input DMAs
            # so the qk chunks run at 2.4 GHz (output never read)
            wps = pp.tile([128, 2, 512], f32, tag="pp", name="warm")
            for i in range(18):
                nc.tensor.matmul(wps[:, i % 2, :], lhsT=warm_sb[:, 0:128],
                                 rhs=warm_sb[:], start=True, stop=True)
            # lead-in: the two qk chunks pair 0 needs
            qk_chunk(pp, 0)
            qk_chunk(pp, 1)
            # proj work to interleave into the score loops: pair 0 gets
            # qk2,3 + v0-3; pairs 1/2 get qk4,5 / qk6,7 (headroom there)
            proj_sched = {
                0: [lambda m=m: qk_chunk(pp, m) for m in (2, 3)]
                   + [lambda c2=c2: v_chunk(pp, c2) for c2 in range(4)],
                1: [lambda m=m: qk_chunk(pp, m) for m in (4, 5)],
                2: [lambda m=m: qk_chunk(pp, m) for m in (6, 7)],
                3: [],
            }

            with tc.tile_pool(name="scA", bufs=1, space="PSUM") as scA, \
                 tc.tile_pool(name="scB", bufs=1, space="PSUM") as scB:
                for t in range(4):
                    uA = upool.tile([128, 8, N], b16, tag="U", bufs=4,
                                    name=f"u{2 * t}")
                    uB = upool.tile([128, 8, N], b16, tag="U", bufs=4,
                                    name=f"u{2 * t + 1}")
                    u_tiles[t] = (uA, uB)
                    res_pairs[t] = [None, None]
                    sched = proj_sched[t]
                    slots = (range(len(sched)) if t == 0
                             else [3, 6][:len(sched)])
                    for jc in range(8):
                        score_step(scA, scB, t, jc, uA, uB)
                        if jc in slots:
                            sched[slots.index(jc) if t else jc]()
                        if t > 0:
                            puA, puB = u_tiles[t - 1]
                            av_step(pp, t - 1, jc, puA, puB, res_pairs[t - 1])
                    if t > 0:
                        evict_pair(t - 1, res_pairs[t - 1])
                        normalize_pair(t - 1)

            # pair 3 attention + out projection (op reuses scA/scB's banks).
            # av3 runs first so the PE never stalls on the op-pool bank WAR
            # (out matmuls wait for the last score ACTs).
            with tc.tile_pool(name="op", bufs=2, space="PSUM") as op:
                psd = {}
                uA, uB = u_tiles[3]
                for jc in range(8):
                    av_step(pp, 3, jc, uA, uB, res_pairs[3])
                evict_pair(3, res_pairs[3])
                out_chunk(op, 0, [0, 1, 2], psd)
                out_chunk(op, 1, [0, 1, 2], psd)
                normalize_pair(3, use_scalar=True)
                out_chunk(op, 0, [3], psd)
                out_finish(0, psd)
                out_chunk(op, 1, [3], psd)
                out_finish(1, psd)
                out_chunk(op, 2, [0, 1, 2, 3], psd)
                out_finish(2, psd)
                out_chunk(op, 3, [0, 1, 2, 3], psd)
                out_finish(3, psd)

    nc.compile()
    return nc


# ------------------------------------------------------------- SPMD dispatch
def _make_spmd_fn(nc, n_cores):
    """bass NEFF runner over axon PJRT WITHOUT buffer donation (donation
    hangs the axon backend)."""
    import jax
    import jax.core
    from jax.sharding import Mesh, PartitionSpec
    from jax.experimental.shard_map import shard_map
    from concourse import mybir
    from concourse.bass2jax import _bass_exec_p, install_neuronx_cc_hook

    install_neuronx_cc_hook()

    partition_name = nc.partition_id_tensor.name if nc.partition_id_tensor else None
    in_names, out_names, out_avals = [], [], []
    for alloc in nc.m.functions[0].allocations:
        if not isinstance(alloc, mybir.MemoryLocationSet):
            continue
        name = alloc.memorylocations[0].name
        if alloc.kind == "ExternalInput":
            if name != partition_name:
                in_names.append(name)
        elif alloc.kind == "ExternalOutput":
            out_names.append(name)
            out_avals.append(jax.core.ShapedArray(
                tuple(alloc.tensor_shape), mybir.dt.np(alloc.dtype)))

    n_params = len(in_names)
    all_in_names = list(in_names) + list(out_names)
    if partition_name is not None:
        all_in_names.append(partition_name)
    zero_outs = [np.zeros(a.shape, a.dtype) for a in out_avals]

    def _body(*args):
        operands = list(args)
        if partition_name is not None:
            from concourse.bass2jax import partition_id_tensor
            operands.append(partition_id_tensor())
        return tuple(_bass_exec_p.bind(
            *operands,
            out_avals=tuple(out_avals),
            in_names=tuple(all_in_names),
            out_names=tuple(out_names),
            lowering_input_output_aliases=(),
            sim_require_finite=True,
            sim_require_nnan=True,
            nc=nc,
        ))

    devices = jax.devices()[:n_cores]
    mesh = Mesh(np.asarray(devices), ("core",))
    sharded = jax.jit(
        shard_map(_body, mesh=mesh,
                  in_specs=(PartitionSpec("core"),) * (n_params + len(out_names)),
                  out_specs=(PartitionSpec("core"),) * len(out_names),
                  check_rep=False),
        keep_unused=True)

    def run(in_maps):
        per_core = [[np.asarray(m[k]) for k in in_names] for m in in_maps]
        concat = [np.concatenate([per_core[c][i] for c in range(n_cores)], axis=0)
                  for i in range(n_params)]
        concat += [np.concatenate([z] * n_cores, axis=0) for z in zero_outs]
        outs = [np.asarray(o) for o in sharded(*concat)]
        results = []
        for c in range(n_cores):
            m = {}
            for i, name in enumerate(out_names):
                rows = out_avals[i].shape[0]
                m[name] = outs[i][c * rows:(c + 1) * rows]
            results.append(m)
        return results

    return run


# ------------------------------------------------------------------ host prep
def _prep_weights(w_proj, b_proj, w_out, b_out):
    # permuted qk columns: chunk m (128 cols): pair t=m//2; m even -> q, odd -> k
    perm = np.empty(1024, np.int64)
    scale = np.empty(1024, np.float32)
    for m in range(8):
        t, is_k = m // 2, m % 2
        for p in range(128):
            h = 2 * t + (1 if p >= 64 else 0)
            d = p % 64
            perm[m * 128 + p] = h * 192 + 64 * is_k + d
            scale[m * 128 + p] = 1.0 if is_k else SCALE
    wqk = (w_proj[:, perm] * scale[None, :]).astype(bf16)
    bqk = (b_proj[perm] * scale).astype(np.float32).reshape(8, 128).T.copy()

    vperm = np.array([(j // 64) * 192 + 128 + (j % 64) for j in range(512)],
                     np.int64)
    wv = w_proj[:, vperm].astype(bf16)
    bvb = np.broadcast_to(b_proj[vperm].astype(np.float32), (128, 512)).copy()

    wo = w_out.astype(bf16)
    bo = b_out.astype(np.float32).reshape(4, 128).T.copy()
    return wqk, bqk, wv, bvb, wo, bo


def kernel(x, w_proj, b_proj, w_out, b_out):
    global _cached_run
    x = np.asarray(x, np.float32)
    w_proj = np.asarray(w_proj, np.float32)
    b_proj = np.asarray(b_proj, np.float32)
    w_out = np.asarray(w_out, np.float32)
    b_out = np.asarray(b_out, np.float32)

    global _cached_nc
    if _cached_run is None:
        nc = _build_nc()
        _cached_nc = nc
        _cached_run = _make_spmd_fn(nc, B)

    wqk, bqk, wv, bvb, wo, bo = _prep_weights(w_proj, b_proj, w_out, b_out)
    in_maps = []
    for b in range(B):
        x2d = np.ascontiguousarray(x[b].reshape(C, N))
        in_maps.append(dict(
            xb=x2d.astype(bf16), wqk=wqk, bqk=bqk,
            wv=wv, bvb=bvb, wo=wo, bo=bo))

    res = _cached_run(in_maps)
    out = np.stack([res[b]["out"].reshape(C, 32, 32) for b in range(B)])
    return out.astype(np.float32)


# revision 33
# speedup vs baseline: 1.1759x; 1.0032x over previous
"""AttentionBlock Trainium2 kernel: 8-way batch-parallel over 8 NeuronCores.

Reference computation (per batch element b):
    tokens = x[b].reshape(C, N).T                  # [N, C], N=1024, C=512
    qkv    = tokens @ w_proj + b_proj              # [N, 3*512]
    per head h (8 heads, D=64):
        att  = softmax(q_h @ k_h.T / 8, axis=keys) # [N, N]
        res_h = att @ v_h                          # [N, 64]
    out = res @ w_out + b_out + tokens             # [N, C]
    return out.T.reshape(C, 32, 32)

Kernel strategy (per core, one batch element), ScalarE-exp-paced pipeline:
  - qk projection transposed: qkT = w_qk.T @ x -> SBUF [d, tokens] (w_proj
    columns host-permuted; head-pair halves in partitions 0-63 / 64-127)
  - v projection direct (v = x.T @ w_v) with per-head ones column at column
    64+(h%2), so attn@v (M=66) accumulates the softmax denominator of the
    even head on PSUM partition 64 and the odd head on partition 65
  - scores transposed scT[j,i] = k.T @ q (row-packed K=64 pairs), exp on
    ScalarE from PSUM; score PSUM pools single-buffered A/B so ScalarE paces
    while PE interleaves attn@v of the previous pair and projections
  - softmax denominators: DVE sum-row copy -> reciprocal_approx_fast per
    pair -> DRAM-bounce partition-broadcast DMA -> DVE multiply
  - out projection outT = w_out.T @ resT, partially overlapped with the last
    pair's attention; residual+bias fused on DVE
  - PSUM (8 banks): shared 2-buf ring "pp" (4) for projections + attn@v res,
    score pools A/B (2+2); tail out-proj pool reuses A/B's banks
  All matmul operands bf16 (fp32 PSUM accumulation).
"""
import sys
sys.path.insert(0, '/opt/trn_rl_repo')

import numpy as np
import ml_dtypes
from contextlib import ExitStack

B, C, N = 8, 512, 1024
NH, D = 8, 64
INNER = NH * D  # 512
SCALE = D ** -0.5

bf16 = ml_dtypes.bfloat16

_cached_run = None
_cached_nc = None


# ---------------------------------------------------------------- bass kernel
def _build_nc():
    import concourse.bass as bass
    import concourse.tile as tile
    from concourse import bacc, mybir

    f32 = mybir.dt.float32
    b16 = mybir.dt.bfloat16
    ts = bass.ts

    nc = bacc.Bacc("TRN2", target_bir_lowering=False, debug=False)

    xb_d = nc.dram_tensor("xb", [C, N], b16, kind="ExternalInput").ap()
    wqk_d = nc.dram_tensor("wqk", [C, 1024], b16, kind="ExternalInput").ap()
    bqk_d = nc.dram_tensor("bqk", [128, 8], f32, kind="ExternalInput").ap()
    wv_d = nc.dram_tensor("wv", [C, 512], b16, kind="ExternalInput").ap()
    bvb_d = nc.dram_tensor("bvb", [128, 512], f32, kind="ExternalInput").ap()
    wo_d = nc.dram_tensor("wo", [INNER, C], b16, kind="ExternalInput").ap()
    bo_d = nc.dram_tensor("bo", [128, 4], f32, kind="ExternalInput").ap()
    recb_d = nc.dram_tensor("recb", [8, N], f32, kind="Internal").ap()
    out_d = nc.dram_tensor("out", [C, N], f32, kind="ExternalOutput").ap()

    with tile.TileContext(nc) as tc, ExitStack() as ctx:
        sb = ctx.enter_context(tc.tile_pool(name="sb", bufs=1))
        upool = ctx.enter_context(tc.tile_pool(name="up", bufs=1))
        rpool = ctx.enter_context(tc.tile_pool(name="rp", bufs=1))

        warm_sb = sb.tile([128, 512], b16)
        nc.gpsimd.memset(warm_sb[:], 0.0)

        # ---- persistent SBUF tensors (input DMAs spread over 3 queues,
        # first-needed first: xb on sync, wqk head-half on scalar, rest gpsimd)
        xb_sb = sb.tile([128, 4, N], b16)
        xbr = xb_d.rearrange("(kc p) n -> p kc n", p=128)
        nc.sync.dma_start(xb_sb[:, 0, :], xbr[:, 0, :])
        nc.sync.dma_start(xb_sb[:, 1, :], xbr[:, 1, :])
        wqk_sb = sb.tile([128, 4, 1024], b16)
        wqkr = wqk_d.rearrange("(kc p) j -> p kc j", p=128)
        nc.scalar.dma_start(wqk_sb[:, :, 0:256], wqkr[:, :, 0:256])
        nc.scalar.dma_start(xb_sb[:, 2, :], xbr[:, 2, :])
        nc.gpsimd.dma_start(xb_sb[:, 3, :], xbr[:, 3, :])
        nc.scalar.dma_start(wqk_sb[:, :, 256:1024], wqkr[:, :, 256:1024])
        bqk_sb = sb.tile([128, 8], f32)
        nc.sync.dma_start(bqk_sb[:], bqk_d[:])
        wv_sb = sb.tile([128, 4, 512], b16)
        nc.gpsimd.dma_start(wv_sb[:], wv_d.rearrange("(kc p) j -> p kc j", p=128))
        bvb_sb = sb.tile([128, 512], f32)
        nc.gpsimd.dma_start(bvb_sb[:], bvb_d[:])
        wo_sb = sb.tile([128, 4, 512], b16)
        nc.sync.dma_start(wo_sb[:], wo_d.rearrange("(kc p) c -> p kc c", p=128))
        bo_sb = sb.tile([128, 4], f32)
        nc.sync.dma_start(bo_sb[:], bo_d[:])

        qkT_sb = sb.tile([128, 8, N], b16)       # [inner%128, qk chunk, token]
        v_sb = sb.tile([128, 8, 8, 96], b16)     # [tok%128, tchunk, h, d|ones|0s]
        resT_sb = sb.tile([128, 4, N], b16)      # [inner%128, pair, token]
        final_sb = sb.tile([128, 4, N], f32)     # [c%128, cchunk, token]
        resU_sb = sb.tile([128, 8, N], b16)      # unnormalized res + sum rows
        tp1_sb = sb.tile([128, 2, N], b16)       # stream-transposed sums
        tp2_sb = sb.tile([128, 2, N], f32)       # strided reciprocals
        tp3_sb = sb.tile([128, 2, N], f32)       # recip rows (back-transposed)
        nc.vector.memset(tp1_sb[:], 1.0)
        nc.vector.memset(tp2_sb[:], 1.0)

        nc.vector.memset(v_sb[:], 0.0)
        nc.vector.memset(v_sb[:, :, :, 64:66], 1.0)  # ones cols -> sum rows
        for cc in range(4):  # final = x + b_out (residual+bias prefill)
            nc.vector.tensor_scalar_add(
                final_sb[:, cc, :], xb_sb[:, cc, :], bo_sb[:, cc, None])

        # ------------------------------------------------ emission helpers
        def qk_chunk(pp, m):
            ps = pp.tile([128, 2, 512], f32, tag="pp", name=f"qk{m}")
            for ih in range(2):
                for kc in range(4):
                    nc.tensor.matmul(
                        ps[:, ih, :],
                        lhsT=wqk_sb[:, kc, ts(m, 128)],
                        rhs=xb_sb[:, kc, ts(ih, 512)],
                        start=(kc == 0), stop=(kc == 3))
            nc.vector.tensor_scalar_add(
                qkT_sb[:, m, :], ps.rearrange("p a b -> p (a b)"),
                bqk_sb[:, m, None])

        def v_chunk(pp, c2):
            ps = pp.tile([128, 2, 512], f32, tag="pp", name=f"v{c2}")
            for half in range(2):
                tch = 2 * c2 + half
                for kc in range(4):
                    nc.tensor.matmul(
                        ps[:, half, :],
                        lhsT=xb_sb[:, kc, ts(tch, 128)],
                        rhs=wv_sb[:, kc, :],
                        start=(kc == 0), stop=(kc == 3))
            for half in range(2):
                nc.vector.tensor_add(
                    v_sb[:, 2 * c2 + half, :, 0:64],
                    ps[:, half, :].rearrange("p (h d) -> p h d", d=64),
                    bvb_sb.rearrange("p (h d) -> p h d", d=64))

        def score_step(scA, scB, t, jc, uA, uB):
            """Score matmuls + exp for (pair t, key chunk jc)."""
            qc, kc = 2 * t, 2 * t + 1
            sA = scA.tile([128, 2, 512], f32, tag="scA", bufs=1,
                          name=f"sA{t}_{jc}")
            sB = scB.tile([128, 2, 512], f32, tag="scB", bufs=1,
                          name=f"sB{t}_{jc}")
            for ih in range(2):  # A/B interleaved: disjoint row groups overlap
                nc.tensor.matmul(
                    sA[:, ih, :],
                    lhsT=qkT_sb[0:64, kc, ts(jc, 128)],
                    rhs=qkT_sb[0:64, qc, ts(ih, 512)],
                    start=True, stop=True)
                nc.tensor.matmul(
                    sB[:, ih, :],
                    lhsT=qkT_sb[64:128, kc, ts(jc, 128)],
                    rhs=qkT_sb[64:128, qc, ts(ih, 512)],
                    start=True, stop=True)
            nc.scalar.activation(
                uA[:, jc, :], sA.rearrange("p a b -> p (a b)"),
                mybir.ActivationFunctionType.Exp)
            nc.scalar.activation(
                uB[:, jc, :], sB.rearrange("p a b -> p (a b)"),
                mybir.ActivationFunctionType.Exp)

        def av_step(pp, t, jc, uA, uB, res2):
            """attn@v accumulation matmuls for pair t, key chunk jc."""
            for half in range(2):
                h = 2 * t + half
                u = uA if half == 0 else uB
                if jc == 0:
                    res2[half] = pp.tile([128, 2, 512], f32, tag="pp",
                                         name=f"res{h}")
                for ih in range(2):
                    nc.tensor.matmul(
                        res2[half][0:96, ih, :],
                        lhsT=v_sb[:, jc, h, :],
                        rhs=u[:, jc, ts(ih, 512)],
                        start=(jc == 0), stop=(jc == 7))

        def evict_pair(t, res2):
            """Evict raw res (with sum rows) to SBUF, freeing the PSUM ring."""
            for half in range(2):
                h = 2 * t + half
                nc.vector.tensor_copy(
                    resU_sb[0:96, h, :],
                    res2[half][0:96].rearrange("p a b -> p (a b)"))

        def normalize_pair(t, use_scalar=False):
            """Reciprocal of the pair's sum rows, broadcast, multiply."""
            if use_scalar:
                # ScalarE exp(-ln(s)) straight onto the sum rows (fast tail)
                nc.scalar.activation(
                    tp2_sb[64:65, :, :].rearrange("p a b -> p (a b)"),
                    resU_sb[64:65, 2 * t:2 * t + 2, :].rearrange(
                        "p a b -> p (a b)"),
                    mybir.ActivationFunctionType.Ln)
                nc.scalar.activation(
                    tp3_sb[64:65, :, :].rearrange("p a b -> p (a b)"),
                    tp2_sb[64:65, :, :].rearrange("p a b -> p (a b)"),
                    mybir.ActivationFunctionType.Exp, scale=-1.0)
            else:
                # DVE 32x32 stream transpose -> strided reciprocal -> back
                nc.vector.transpose(
                    tp1_sb[64:96, :, :], resU_sb[64:96, 2 * t:2 * t + 2, :])
                nc.vector.reciprocal(
                    tp2_sb[64:96, :, :].rearrange(
                        "p a (b o) -> p a b o", o=32)[:, :, :, 0],
                    tp1_sb[64:96, :, :].rearrange(
                        "p a (b o) -> p a b o", o=32)[:, :, :, 0])
                nc.vector.transpose(tp3_sb[64:96, :, :], tp2_sb[64:96, :, :])
            for half in range(2):
                h = 2 * t + half
                nc.gpsimd.dma_start(recb_d[h:h + 1, :], tp3_sb[64:65, half, :])
                bc = rpool.tile([64, N], f32, tag="bc", bufs=4, name=f"bc{h}")
                nc.gpsimd.dma_start(
                    bc[:], recb_d[h:h + 1, :].broadcast_to([64, N]))
                if half == 0:
                    nc.vector.tensor_mul(
                        resT_sb[0:64, t, :], resU_sb[0:64, h, :], bc[:])
                else:
                    tmp = rpool.tile([64, N], b16, tag="tmpod", bufs=2,
                                     name=f"tm{h}")
                    nc.vector.tensor_mul(
                        tmp[:], resU_sb[0:64, h, :], bc[:])
                    nc.sync.dma_start(resT_sb[64:128, t, :], tmp[:])

        def out_chunk(op, cc, kcs, psd):
            """Out-projection accumulation over pair chunks kcs for chunk cc."""
            if 0 in kcs:
                psd[cc] = op.tile([128, 2, 512], f32, tag="op", name=f"o{cc}")
            for ih in range(2):
                for kc in kcs:
                    nc.tensor.matmul(
                        psd[cc][:, ih, :],
                        lhsT=wo_sb[:, kc, ts(cc, 128)],
                        rhs=resT_sb[:, kc, ts(ih, 512)],
                        start=(kc == 0), stop=(kc == 3))

        def out_finish(cc, psd):
            nc.vector.tensor_add(
                final_sb[:, cc, :],
                psd[cc].rearrange("p a b -> p (a b)"),
                final_sb[:, cc, :])
            eng = nc.sync if cc % 2 == 0 else nc.gpsimd
            eng.dma_start(
                out_d.rearrange("(cc p) n -> p cc n", p=128)[:, cc, :],
                final_sb[:, cc, :])

        # ------------------------------------------------ pipeline emission
        u_tiles = {}
        res_pairs = {}
        with tc.tile_pool(name="pp", bufs=2, space="PSUM") as pp:
            # HAM warm-up: dummy matmuls keep the PE busy during input DMAs
            # so the qk chunks run at 2.4 GHz (output never read)
            wps = pp.tile([128, 2, 512], f32, tag="pp", name="warm")
            for i in range(40):
                nc.tensor.matmul(wps[:, i % 2, :], lhsT=warm_sb[:, 0:128],
                                 rhs=warm_sb[:], start=True, stop=True)
            # lead-in: the two qk chunks pair 0 needs
            qk_chunk(pp, 0)
            qk_chunk(pp, 1)
            # proj work to interleave into the score loops: pair 0 gets
            # qk2,3 + v0-3; pairs 1/2 get qk4,5 / qk6,7 (headroom there)
            proj_sched = {
                0: [lambda m=m: qk_chunk(pp, m) for m in (2, 3)]
                   + [lambda c2=c2: v_chunk(pp, c2) for c2 in range(4)],
                1: [lambda m=m: qk_chunk(pp, m) for m in (4, 5)],
                2: [lambda m=m: qk_chunk(pp, m) for m in (6, 7)],
                3: [],
            }

            with tc.tile_pool(name="scA", bufs=1, space="PSUM") as scA, \
                 tc.tile_pool(name="scB", bufs=1, space="PSUM") as scB:
                for t in range(4):
                    uA = upool.tile([128, 8, N], b16, tag="U", bufs=4,
                                    name=f"u{2 * t}")
                    uB = upool.tile([128, 8, N], b16, tag="U", bufs=4,
                                    name=f"u{2 * t + 1}")
                    u_tiles[t] = (uA, uB)
                    res_pairs[t] = [None, None]
                    sched = proj_sched[t]
                    slots = (range(len(sched)) if t == 0
                             else [3, 6][:len(sched)])
                    for jc in range(8):
                        score_step(scA, scB, t, jc, uA, uB)
                        if jc in slots:
                            sched[slots.index(jc) if t else jc]()
                        if t > 0:
                            puA, puB = u_tiles[t - 1]
                            av_step(pp, t - 1, jc, puA, puB, res_pairs[t - 1])
                    if t > 0:
                        evict_pair(t - 1, res_pairs[t - 1])
                        normalize_pair(t - 1)

            # pair 3 attention + out projection (op reuses scA/scB's banks).
            # av3 runs first so the PE never stalls on the op-pool bank WAR
            # (out matmuls wait for the last score ACTs).
            with tc.tile_pool(name="op", bufs=2, space="PSUM") as op:
                psd = {}
                uA, uB = u_tiles[3]
                for jc in range(8):
                    av_step(pp, 3, jc, uA, uB, res_pairs[3])
                evict_pair(3, res_pairs[3])
                out_chunk(op, 0, [0, 1, 2], psd)
                out_chunk(op, 1, [0, 1, 2], psd)
                normalize_pair(3)
                out_chunk(op, 0, [3], psd)
                out_finish(0, psd)
                out_chunk(op, 1, [3], psd)
                out_finish(1, psd)
                out_chunk(op, 2, [0, 1, 2, 3], psd)
                out_finish(2, psd)
                out_chunk(op, 3, [0, 1, 2, 3], psd)
                out_finish(3, psd)

    nc.compile()
    return nc


# ------------------------------------------------------------- SPMD dispatch
def _make_spmd_fn(nc, n_cores):
    """bass NEFF runner over axon PJRT WITHOUT buffer donation (donation
    hangs the axon backend)."""
    import jax
    import jax.core
    from jax.sharding import Mesh, PartitionSpec
    from jax.experimental.shard_map import shard_map
    from concourse import mybir
    from concourse.bass2jax import _bass_exec_p, install_neuronx_cc_hook

    install_neuronx_cc_hook()

    partition_name = nc.partition_id_tensor.name if nc.partition_id_tensor else None
    in_names, out_names, out_avals = [], [], []
    for alloc in nc.m.functions[0].allocations:
        if not isinstance(alloc, mybir.MemoryLocationSet):
            continue
        name = alloc.memorylocations[0].name
        if alloc.kind == "ExternalInput":
            if name != partition_name:
                in_names.append(name)
        elif alloc.kind == "ExternalOutput":
            out_names.append(name)
            out_avals.append(jax.core.ShapedArray(
                tuple(alloc.tensor_shape), mybir.dt.np(alloc.dtype)))

    n_params = len(in_names)
    all_in_names = list(in_names) + list(out_names)
    if partition_name is not None:
        all_in_names.append(partition_name)
    zero_outs = [np.zeros(a.shape, a.dtype) for a in out_avals]

    def _body(*args):
        operands = list(args)
        if partition_name is not None:
            from concourse.bass2jax import partition_id_tensor
            operands.append(partition_id_tensor())
        return tuple(_bass_exec_p.bind(
            *operands,
            out_avals=tuple(out_avals),
            in_names=tuple(all_in_names),
            out_names=tuple(out_names),
            lowering_input_output_aliases=(),
            sim_require_finite=True,
            sim_require_nnan=True,
            nc=nc,
        ))

    devices = jax.devices()[:n_cores]
    mesh = Mesh(np.asarray(devices), ("core",))
    sharded = jax.jit(
        shard_map(_body, mesh=mesh,
                  in_specs=(PartitionSpec("core"),) * (n_params + len(out_names)),
                  out_specs=(PartitionSpec("core"),) * len(out_names),
                  check_rep=False),
        keep_unused=True)

    def run(in_maps):
        per_core = [[np.asarray(m[k]) for k in in_names] for m in in_maps]
        concat = [np.concatenate([per_core[c][i] for c in range(n_cores)], axis=0)
                  for i in range(n_params)]
        concat += [np.concatenate([z] * n_cores, axis=0) for z in zero_outs]
        outs = [np.asarray(o) for o in sharded(*concat)]
        results = []
        for c in range(n_cores):
            m = {}
            for i, name in enumerate(out_names):
                rows = out_avals[i].shape[0]
                m[name] = outs[i][c * rows:(c + 1) * rows]
            results.append(m)
        return results

    return run


# ------------------------------------------------------------------ host prep
def _prep_weights(w_proj, b_proj, w_out, b_out):
    # permuted qk columns: chunk m (128 cols): pair t=m//2; m even -> q, odd -> k
    perm = np.empty(1024, np.int64)
    scale = np.empty(1024, np.float32)
    for m in range(8):
        t, is_k = m // 2, m % 2
        for p in range(128):
            h = 2 * t + (1 if p >= 64 else 0)
            d = p % 64
            perm[m * 128 + p] = h * 192 + 64 * is_k + d
            scale[m * 128 + p] = 1.0 if is_k else SCALE
    wqk = (w_proj[:, perm] * scale[None, :]).astype(bf16)
    bqk = (b_proj[perm] * scale).astype(np.float32).reshape(8, 128).T.copy()

    vperm = np.array([(j // 64) * 192 + 128 + (j % 64) for j in range(512)],
                     np.int64)
    wv = w_proj[:, vperm].astype(bf16)
    bvb = np.broadcast_to(b_proj[vperm].astype(np.float32), (128, 512)).copy()

    wo = w_out.astype(bf16)
    bo = b_out.astype(np.float32).reshape(4, 128).T.copy()
    return wqk, bqk, wv, bvb, wo, bo


def kernel(x, w_proj, b_proj, w_out, b_out):
    global _cached_run
    x = np.asarray(x, np.float32)
    w_proj = np.asarray(w_proj, np.float32)
    b_proj = np.asarray(b_proj, np.float32)
    w_out = np.asarray(w_out, np.float32)
    b_out = np.asarray(b_out, np.float32)

    global _cached_nc
    if _cached_run is None:
        nc = _build_nc()
        _cached_nc = nc
        _cached_run = _make_spmd_fn(nc, B)

    wqk, bqk, wv, bvb, wo, bo = _prep_weights(w_proj, b_proj, w_out, b_out)
    in_maps = []
    for b in range(B):
        x2d = np.ascontiguousarray(x[b].reshape(C, N))
        in_maps.append(dict(
            xb=x2d.astype(bf16), wqk=wqk, bqk=bqk,
            wv=wv, bvb=bvb, wo=wo, bo=bo))

    res = _cached_run(in_maps)
    out = np.stack([res[b]["out"].reshape(C, 32, 32) for b in range(B)])
    return out.astype(np.float32)
